# revision 27
# baseline (speedup 1.0000x reference)
"""BinHD Hamming-distance kernel for 8 Trainium2 NeuronCores.

dist[n, c] = sum_d xor(samples[n, d], classes_hv[c, d])
           = s_sum[n] + c_sum[c] - 2 * (samples @ classes_hv.T)[n, c]

Strategy (data-parallel over samples):
  - shard samples row-wise across 8 cores (1024 rows each); replicate classes.
  - per core: a [1024 x 9984] x [9984 x 1000] GEMM on the TensorEngine in
    fp8e4m3 with perf_mode=DoubleRow (2 MACs/cell/cycle). Inputs are {0,1} and
    {0,-2} -> fp8 is exact; PSUM accumulates fp32 -> the cross term is exact.
  - classes are pre-scaled by -2 so PSUM directly holds -2*cross; the epilogue
    is a single DVE add of a host-precomputed bias plane
    bias[n, c] = s_sum[n] + c_sum[c] - 2 * samples[n, 9984:] @ classes[c, 9984:]
    (the K remainder 10000 = 39*256 + 16 is folded into the bias on the host,
    saving a full super-tile of matmuls on the PE). out is bf16 (abs err <= 16
    on ~5000-scale values, rel ~3.4e-3 vs the 2e-2 gate); bias stays f32.

Perf structure:
  - All loads ride ONE HWDGE ring (nc.sync) in exact consumption order, so the
    ring FIFO acts as a priority queue: the PE's next operands always get the
    full HBM bandwidth. Stores ride the other ring (nc.scalar) so their DVE
    waits never head-of-line-block a load.
  - Pass 0's operands are host-packed into per-supertile "superchunks"
    [bt_t | at_t(m0..m3)] = one 386KB transfer per k-supertile (singles first,
    then groups of 2/3/4), so exactly one DMA completion gates each k-step.
  - Two fp32 dummy matmuls on framework const APs (no cross-engine deps) warm
    the PE HAM clock gate during the runtime preamble, so real matmuls run at
    2.4 GHz from (nearly) the first one regardless of HAM window phase.
  - m-tiles run in k-passes of [4, 3, 1] tiles with explicitly bank-assigned
    PSUM (consecutive groups overlap banks so the next pass's matmuls start
    while the previous epilogue drains). at for m4..m7 + bias planes stream
    during pass 0/1 behind the superchunks on the same ring.

DoubleRow layout: each matmul contracts K=256 via 3D APs [p, i, free] with
k = 256*t + 128*i + p (planar i-major packing in SBUF).
"""

import sys

if "/opt/trn_rl_repo" not in sys.path:
    sys.path.insert(0, "/opt/trn_rl_repo")

import numpy as np
import ml_dtypes

N, D, C = 8192, 10000, 1000
N_CORES = 8
P = 128
TT = 39                  # k-super-tiles of 256 on the PE (covers 9984 of D)
K_MM = TT * 2 * P        # 9984
C_PAD = 1008             # classes padded 1000 -> 1008 (512 + 496 psum chunks)
NQ = 2
QSTRIDE = [512, 496]     # SBUF i-plane strides (DoubleRow: stride % 16 == 0)
QW = [512, 488]          # streamed widths; q1 streams 488 of its 496 plane
BT_B = 2 * (QSTRIDE[0] + QSTRIDE[1])   # 2016 bt bytes per supertile/partition
CK_B = BT_B + 4 * 256                  # 3040 superchunk bytes (bt + at m0..3)
M_SH = N // N_CORES      # 1024 sample rows per core
MT = M_SH // P           # 8 m-tiles per core
AT_HALF = (TT // 2) * 256              # 4864: at half-split point (19 tiles)

M_GROUPS = [[0, 1, 2, 3], [4, 5, 6], [7]]
BANKS = [
    [0, 1, 2, 3, 4, 5, 6, 7],  # group 0
    [0, 1, 2, 3, 4, 5],        # group 1 (0,1 freed first by g0's epilogue)
    [6, 7],                    # group 2
]

# superchunk transfer groups: singles while the PE ramps, then 2/3/4s
CK_SIZES = [1, 1, 1, 1, 1, 1, 2, 3, 4, 4, 4, 4, 4, 4, 4]
CK_STARTS = np.cumsum([0] + CK_SIZES).tolist()
assert CK_STARTS[-1] == TT

# extra loads woven into the ring after a given ck-group index (pass 0),
# sized so the superchunk stream never falls behind the PE.
EXTRAS_P0 = {8: ["m4a"], 9: ["m5a"], 10: ["m6a"],
             11: ["b0"], 12: ["b1"], 13: ["b2"], 14: ["b3"]}
EXTRAS_P1 = ["m4b", "m5b", "m6b", "m7a", "b4", "m7b", "b5", "b6"]
EXTRAS_P2 = ["b7"]

DUMMY_FDS = [512, 256]   # dummy fp32 matmuls (each lowers to a hi+lo pass,
                         # ~2.1us / ~1.0us cold) that warm the PE clock gate;
                         # sized to span busy time from the post-barrier PE
                         # start (~7.2us) to first-operand-ready (~10.3us)

F8 = ml_dtypes.float8_e4m3

_compiled = None


def _build():
    import concourse.mybir as mybir
    from concourse import bacc
    from concourse.tile import TileContext

    nc = bacc.Bacc("TRN2", target_bir_lowering=False, debug=False)
    f8 = mybir.dt.float8e4
    f32 = mybir.dt.float32
    bf16 = mybir.dt.bfloat16
    DR = mybir.MatmulPerfMode.DoubleRow

    # ck: [p, (t ck_b)] superchunks: per supertile bt (2016) + at m0..3 (1024)
    ck_d = nc.declare_dram_parameter("ck", [P, TT * CK_B], f8, isOutput=False)
    # at4: samplesT for m-tiles 4..7, per-partition contiguous [p, (t i mcol)]
    at4_d = nc.declare_dram_parameter("at4", [4, P, TT * 256], f8, isOutput=False)
    bias_d = nc.declare_dram_parameter("bias", [MT, P, C_PAD], f32, isOutput=False)
    out_d = nc.declare_dram_parameter("out", [MT, P, C_PAD], bf16, isOutput=True)

    with TileContext(nc) as tc:
        with (
            tc.tile_pool(name="ckp", bufs=1) as ckp,
            tc.tile_pool(name="atp", bufs=1) as atp,
            tc.tile_pool(name="pp", bufs=1, space="PSUM") as pp,
            tc.tile_pool(name="op", bufs=3) as op,
            tc.tile_pool(name="bp", bufs=1) as bp,
        ):
            # -- PE warm-up: HAM-gate flips to 2.4 GHz after ~3.4us of
            # sustained matmul activity; burn that in while the runtime
            # preamble + first loads are still in flight. The operands are
            # framework const APs (materialized during Bacc init, before the
            # pool barrier), so the first dummy issues the moment the PE
            # clears the barrier -- no cross-engine dependency. fp32 runs at
            # quarter rate, which is exactly what a time-burner wants. The
            # dummy PSUM bank is overwritten by the first real start=True
            # matmul on its bank.
            dwc = nc.const_aps.tensor(0.0, (P, 128), f32)
            dps = pp.tile([P, 512], f32, tag="bank7", name="dummy_ps")
            for fd in DUMMY_FDS:
                dxc = nc.const_aps.tensor(0.0, (P, fd), f32)
                nc.tensor.matmul(
                    dps[:, 0:fd], dwc, dxc, start=True, stop=True
                )

            # resident operand tiles
            at_t = {}    # m-tile -> full-K at tile (m4..7)
            bias_t = {}  # m-tile -> bias tile

            def issue_extra(name):
                if name[0] == "m":
                    m = int(name[1])
                    if m not in at_t:
                        at_t[m] = atp.tile(
                            [P, TT * 256], f8, tag=f"atm{m}", name=f"atm{m}"
                        )
                    lo, hi = (0, AT_HALF) if name[2] == "a" else (AT_HALF, TT * 256)
                    nc.sync.dma_start(
                        out=at_t[m][:, lo:hi], in_=at4_d[m - 4, :, lo:hi]
                    )
                else:
                    m = int(name[1])
                    bias_t[m] = bp.tile(
                        [P, C_PAD], f32, tag=f"bias{m}", name=f"bias{m}"
                    )
                    nc.sync.dma_start(out=bias_t[m], in_=bias_d[m])

            cks = [None] * len(CK_SIZES)

            for gi, mgroup in enumerate(M_GROUPS):
                nm = len(mgroup)
                ps = [
                    [
                        pp.tile(
                            [P, QW[q]], f32,
                            tag=f"bank{BANKS[gi][2 * li + q]}",
                            name=f"ps_g{gi}_m{li}_q{q}",
                        )
                        for q in range(NQ)
                    ]
                    for li in range(nm)
                ]
                if gi == 1:
                    for name in EXTRAS_P1:
                        issue_extra(name)
                if gi == 2:
                    for name in EXTRAS_P2:
                        issue_extra(name)
                if gi == len(M_GROUPS) - 1:
                    # final m-tile: run the whole q0 accumulation first, then
                    # q1, so q0's epilogue+store complete ~8us before the
                    # stream ends and q1's tail drain has the DVE to itself
                    for q in range(NQ):
                        for t in range(TT):
                            g = int(np.searchsorted(CK_STARTS, t, side="right")) - 1
                            base = (t - CK_STARTS[g]) * CK_B
                            lhs3 = at_t[mgroup[0]][
                                :, t * 256:(t + 1) * 256
                            ].rearrange("p (i m) -> p i m", i=2)
                            qb = base + q * 2 * QSTRIDE[0]
                            rhs3 = cks[g][
                                :, qb:qb + 2 * QSTRIDE[q]
                            ].rearrange("p (i n) -> p i n", i=2)[:, :, 0:QW[q]]
                            nc.tensor.matmul(
                                ps[0][q], lhs3, rhs3,
                                start=(t == 0), stop=(t == TT - 1),
                                perf_mode=DR,
                            )
                for t in range(TT if gi < len(M_GROUPS) - 1 else 0):
                    g = int(np.searchsorted(CK_STARTS, t, side="right")) - 1
                    j = t - CK_STARTS[g]
                    if gi == 0 and j == 0:
                        ckt = ckp.tile(
                            [P, CK_SIZES[g] * CK_B], f8,
                            tag=f"ck{g}", name=f"ck{g}",
                        )
                        if g == 0:
                            # split the very first chunk across both HWDGE
                            # rings, finest pieces first: the t=0 q0 matmuls
                            # need only the q0 bt planes + the at block
                            nc.sync.dma_start(
                                out=ckt[:, 0:2 * QSTRIDE[0]],
                                in_=ck_d[:, 0:2 * QSTRIDE[0]],
                            )
                            nc.scalar.dma_start(
                                out=ckt[:, BT_B:CK_B], in_=ck_d[:, BT_B:CK_B]
                            )
                            nc.sync.dma_start(
                                out=ckt[:, 2 * QSTRIDE[0]:BT_B],
                                in_=ck_d[:, 2 * QSTRIDE[0]:BT_B],
                            )
                        else:
                            nc.sync.dma_start(
                                out=ckt,
                                in_=ck_d[
                                    :, CK_STARTS[g] * CK_B:CK_STARTS[g + 1] * CK_B
                                ],
                            )
                        cks[g] = ckt
                        for name in EXTRAS_P0.get(g, []):
                            issue_extra(name)
                    base = j * CK_B
                    # q-major at the very first supertile so the four q0
                    # matmuls start as soon as the first ck0 piece lands
                    order = (
                        [(li, q) for q in range(NQ) for li in range(nm)]
                        if (gi == 0 and t == 0)
                        else [(li, q) for li in range(nm) for q in range(NQ)]
                    )
                    for li, q in order:
                        m = mgroup[li]
                        if gi == 0:
                            lhs3 = cks[g][
                                :, base + BT_B + li * 256:base + BT_B + (li + 1) * 256
                            ].rearrange("p (i m) -> p i m", i=2)
                        else:
                            lhs3 = at_t[m][
                                :, t * 256:(t + 1) * 256
                            ].rearrange("p (i m) -> p i m", i=2)
                        qb = base + q * 2 * QSTRIDE[0]
                        rhs3 = cks[g][
                            :, qb:qb + 2 * QSTRIDE[q]
                        ].rearrange("p (i n) -> p i n", i=2)[:, :, 0:QW[q]]
                        nc.tensor.matmul(
                            ps[li][q], lhs3, rhs3,
                            start=(t == 0), stop=(t == TT - 1),
                            perf_mode=DR,
                        )
                for li in range(nm):
                    m = mgroup[li]
                    bt = bias_t[m]
                    o = op.tile([P, C_PAD], bf16)
                    last = gi == len(M_GROUPS) - 1
                    nc.vector.tensor_add(o[:, 0:512], ps[li][0][:], bt[:, 0:512])
                    nc.scalar.dma_start(out=out_d[m, :, 0:512], in_=o[:, 0:512])
                    if not last:
                        nc.vector.tensor_add(
                            o[:, 512:512 + QW[1]], ps[li][1][:],
                            bt[:, 512:512 + QW[1]],
                        )
                        nc.scalar.dma_start(
                            out=out_d[m, :, 512:512 + QW[1]],
                            in_=o[:, 512:512 + QW[1]],
                        )
                    else:
                        # very last m-tile: drain q1 in two chunks across both
                        # rings. The split is asymmetric (384/104): the DVE add
                        # runs ~1.6ns/col, so a small last chunk finishes its
                        # add early and its store's ~1.3us HWDGE dispatch
                        # starts sooner, balancing the two store chains.
                        h = 352
                        nc.vector.tensor_add(
                            o[:, 512:512 + h], ps[li][1][:, 0:h],
                            bt[:, 512:512 + h],
                        )
                        nc.sync.dma_start(
                            out=out_d[m, :, 512:512 + h], in_=o[:, 512:512 + h]
                        )
                        nc.vector.tensor_add(
                            o[:, 512 + h:512 + QW[1]],
                            ps[li][1][:, h:QW[1]],
                            bt[:, 512 + h:512 + QW[1]],
                        )
                        nc.scalar.dma_start(
                            out=out_d[m, :, 512 + h:512 + QW[1]],
                            in_=o[:, 512 + h:512 + QW[1]],
                        )

    nc.compile()
    return nc


def _prep_inputs(samples: np.ndarray, classes_hv: np.ndarray):
    """Host-side shard + layout prep. All values stay exactly representable."""
    samples = np.ascontiguousarray(samples, dtype=np.float32)
    classes_hv = np.ascontiguousarray(classes_hv, dtype=np.float32)

    s_sum = samples.sum(axis=1, dtype=np.float32)        # [N], ints <= D
    c_sum = classes_hv.sum(axis=1, dtype=np.float32)     # [C]
    c_pad = np.zeros(C_PAD, np.float32)
    c_pad[:C] = c_sum
    bias_full = s_sum[:, None] + c_pad[None, :]          # [N, C_PAD] f32
    # K remainder (d >= 9984) folded into the bias plane (exact int math)
    bias_full[:, :C] += (-2.0 * samples[:, K_MM:]) @ classes_hv[:, K_MM:].T

    # bt: (-2*classes).T per supertile, i-major planar [p, (q i n)]
    B8 = np.zeros((K_MM, C_PAD), F8)
    B8[:, :C] = (-2.0 * classes_hv[:, :K_MM]).astype(F8).T
    b5 = B8.reshape(TT, 2, P, C_PAD)                     # [t, i, p, n]
    b0 = (
        b5[:, :, :, :QSTRIDE[0]].transpose(2, 0, 1, 3).reshape(P, TT, 2 * QSTRIDE[0])
    )
    b1 = (
        b5[:, :, :, QSTRIDE[0]:].transpose(2, 0, 1, 3).reshape(P, TT, 2 * QSTRIDE[1])
    )

    in_maps = []
    for core in range(N_CORES):
        rows = slice(core * M_SH, (core + 1) * M_SH)
        A8 = samples[rows, :K_MM].astype(F8).T           # [K_MM, 1024]
        # [k, m] -> [m-tile, p, (t i mcol)]
        at_c = np.ascontiguousarray(
            A8.reshape(TT, 2, P, MT, P)                  # [t, i, p, mt, m]
            .transpose(3, 2, 0, 1, 4)                    # [mt, p, t, i, m]
            .reshape(MT, P, TT * 256)
        )
        # superchunks: [p, t, (bt | at m0..3)] -> [p, (t ck)]
        at03 = at_c[:4].reshape(4, P, TT, 256).transpose(1, 2, 0, 3)  # [p,t,4,256]
        ck = np.concatenate(
            [b0, b1, at03.reshape(P, TT, 4 * 256)], axis=2
        ).reshape(P, TT * CK_B)
        bias_c = np.ascontiguousarray(bias_full[rows].reshape(MT, P, C_PAD))
        in_maps.append({
            "ck": np.ascontiguousarray(ck),
            "at4": np.ascontiguousarray(at_c[4:]),
            "bias": bias_c,
        })
    return in_maps


def _run(inputs: dict, trace: bool = False, **spmd_kwargs):
    from concourse.bass_utils import run_bass_kernel_spmd

    global _compiled
    if _compiled is None:
        _compiled = _build()

    in_maps = _prep_inputs(inputs["samples"], inputs["classes_hv"])
    res = run_bass_kernel_spmd(
        _compiled, in_maps, list(range(N_CORES)), trace=trace, **spmd_kwargs
    )
    parts = [
        res.results[c]["out"].reshape(M_SH, C_PAD)[:, :C].astype(np.float32)
        for c in range(N_CORES)
    ]
    out = np.concatenate(parts, axis=0)
    return out, res


def kernel(samples: np.ndarray, classes_hv: np.ndarray) -> np.ndarray:
    out, _ = _run({"samples": samples, "classes_hv": classes_hv})
    return out
